# revision 14
# baseline (speedup 1.0000x reference)
"""Trainium2 Bass kernel for a 4-layer binarized MLP (BNN) in eval mode.

Network (B=16384, D_in=784, H=4096, D_out=10), all matmuls use sign(w):
  h1 = hardtanh(BN1(x @ sign(w1).T + b1))
  h2 = hardtanh(BN2(sign(h1) @ sign(w2).T + b2))
  h3 = hardtanh(BN3(sign(h2) @ sign(w3).T + b3))
  out = log_softmax(sign(h3) @ sign(w4).T + b4)

Key observations used here:
  * Only the SIGN of h1/h2/h3 matters downstream (hardtanh preserves sign),
    so each hidden layer reduces to  a_out = Sign(psum * s + c)  with
    s = g*rsqrt(v+eps), c = (b-m)*s + be  folded on the host.
  * sign() values are exactly representable in fp8e4/bf16, and matmuls of
    +-1 values accumulate exactly in fp32 PSUM -> layers 2..4 are exact.
  * Layer 1 needs ~fp32 precision on x: x is split into two fp16 terms
    (hi+lo captures ~22 mantissa bits; PE handles fp16 subnormals exactly).
    Both terms are CONCATENATED along K (with sign(w1).T stacked twice) so
    layer 1 is one [1664, B] x [1664, 4096] matmul accumulated in PSUM.
  * Data-parallel over 8 cores: batch is sharded 8 x 2048; weights are
    binarized+transposed+pre-tiled on the host and replicated.

Layout on device (feature-major activations; batch on the free dim):
  activations a_l : SBUF [128, 32 ktiles, B_CHUNK]   (fp8e4 +-1)
  weights W_l^T   : DRAM [32 jtiles, 128 kp, ktiles, 128 cols], streamed
                    per j-tile; matmul lhsT = wt[:, kt, :]  (stationary)
  psum            : [128, B_CHUNK] fp32, accumulated over ktiles
Final layer produces y4 [16, B] (10 valid rows), PE-transposed in 128-col
chunks to [128, 16], then log_softmax along the free dim and DMA to the
output [B_shard, 10].
"""

import numpy as np
import ml_dtypes

# ---- problem constants (hardcoded per the harness contract) ----
B, D_IN, H, OUT = 16384, 784, 4096, 10
N_CORES = 8
BS = B // N_CORES          # 2048 rows per core
BC = 1024                  # batch chunk processed per pass
NB = BS // BC              # chunks per core
KD = 13                    # 1664 = 13*128 k-tiles: [hi;lo] fp16 concat (2*784 padded)
KH = H // 128              # 32 k-tiles for hidden layers
JT = H // 128              # 32 output-channel tiles
N0 = 512                   # matmul moving free-dim chunk
OP = 16                    # padded output channels (10 -> 16)
BN_EPS = 1e-5

_BF16 = ml_dtypes.bfloat16

_compiled = None  # cache of (nc, run_fn)


def _build_module():
    import concourse.bass as bass
    import concourse.tile as tile
    from concourse import bacc, mybir
    from concourse.masks import make_identity
    from contextlib import ExitStack

    f32 = mybir.dt.float32
    bf16 = mybir.dt.bfloat16
    fp16 = mybir.dt.float16
    fp8 = mybir.dt.float8e4
    AF = mybir.ActivationFunctionType
    AX = mybir.AxisListType

    nc = bacc.Bacc("TRN2", target_bir_lowering=False, debug=False,
                   num_devices=N_CORES)

    dr = {}
    dr["xt"] = nc.dram_tensor("xt", [128, KD, BS], fp16, kind="ExternalInput").ap()
    dr["w1"] = nc.dram_tensor("w1", [JT, 128, KD, 128], fp16, kind="ExternalInput").ap()
    dr["w2"] = nc.dram_tensor("w2", [JT, 128, KH, 128], fp8, kind="ExternalInput").ap()
    dr["w3"] = nc.dram_tensor("w3", [JT, 128, KH, 128], fp8, kind="ExternalInput").ap()
    dr["w4"] = nc.dram_tensor("w4", [128, KH, OP], fp8, kind="ExternalInput").ap()
    for l in (1, 2, 3):
        dr[f"sc{l}"] = nc.dram_tensor(f"sc{l}", [128, JT], f32, kind="ExternalInput").ap()
        dr[f"cc{l}"] = nc.dram_tensor(f"cc{l}", [128, JT], f32, kind="ExternalInput").ap()
    dr["b4"] = nc.dram_tensor("b4", [OP, 1], f32, kind="ExternalInput").ap()
    out_dram = nc.dram_tensor("out", [BS, OUT], f32, kind="ExternalOutput").ap()

    with tile.TileContext(nc) as tc, ExitStack() as ctx:
        const = ctx.enter_context(tc.tile_pool(name="const", bufs=1))
        xt_pool = ctx.enter_context(tc.tile_pool(name="xt", bufs=1))
        w1pool = ctx.enter_context(tc.tile_pool(name="w1p", bufs=3))
        wpool = ctx.enter_context(tc.tile_pool(name="wp", bufs=3))
        apool = ctx.enter_context(tc.tile_pool(name="acts", bufs=2))
        small = ctx.enter_context(tc.tile_pool(name="small", bufs=4))
        pspool = ctx.enter_context(tc.tile_pool(name="ps", bufs=2, space="PSUM"))
        ps4pool = ctx.enter_context(tc.tile_pool(name="ps4", bufs=1, space="PSUM"))
        pstpool = ctx.enter_context(tc.tile_pool(name="pst", bufs=2, space="PSUM"))

        def load_xt(b_off):
            # gpsimd issues these so the sync engine can start on weight
            # blocks immediately; per-k-tile tiles so the first matmuls
            # only wait on their own slice
            xt = []
            for kt in range(KD):
                t = xt_pool.tile([128, BC], fp16, tag=f"xtk{kt}",
                                 name=f"xtk{kt}")
                nc.gpsimd.dma_start(out=t,
                                    in_=dr["xt"][:, kt, b_off:b_off + BC])
                xt.append(t)
            return t and xt

        xt0 = load_xt(0)

        # constants (issued on the vector engine; needed only by epilogues)
        s_sb, c_sb = {}, {}
        for l in (1, 2, 3):
            s_sb[l] = const.tile([128, JT], f32, tag=f"s{l}", name=f"s_sb{l}")
            nc.scalar.dma_start(out=s_sb[l], in_=dr[f"sc{l}"])
            c_sb[l] = const.tile([128, JT], f32, tag=f"c{l}", name=f"c_sb{l}")
            nc.scalar.dma_start(out=c_sb[l], in_=dr[f"cc{l}"])
        b4sb = const.tile([OP, 1], f32, tag="b4")
        nc.scalar.dma_start(out=b4sb, in_=dr["b4"])
        w4t = const.tile([128, KH, OP], fp8, tag="w4")
        nc.scalar.dma_start(out=w4t, in_=dr["w4"])
        ident = const.tile([OP, OP], f32, tag="ident")
        make_identity(nc, ident)

        for cb in range(NB):
            b_off = cb * BC
            xt = xt0 if cb == 0 else load_xt(b_off)

            # ---- layer 1: [hi;lo] fp16 concat matmul ----
            a1 = apool.tile([128, KH, BC], fp8, tag="act")
            for j in range(JT):
                wt = w1pool.tile([128, KD, 128], fp16, tag="w1")
                nc.sync.dma_start(out=wt, in_=dr["w1"][j])
                ps = pspool.tile([128, BC], f32, tag="ps")
                for kt in range(KD):
                    lhsT = wt[:, kt, :]
                    for b0 in range(0, BC, N0):
                        nc.tensor.matmul(
                            ps[:, b0:b0 + N0], lhsT,
                            xt[kt][:, b0:b0 + N0],
                            start=(kt == 0), stop=(kt == KD - 1))
                nc.scalar.activation(a1[:, j, :], ps, AF.Sign,
                                     bias=c_sb[1][:, j:j + 1],
                                     scale=s_sb[1][:, j:j + 1])

            # ---- layers 2 and 3: fp8 sign matmuls; layer 4 pair-matmuls
            # are interleaved into layer 3's j-loop (pair t only needs a3
            # j-tiles 2t and 2t+1) so L4 is off the critical path ----
            a_in = a1
            ps4 = ps4pool.tile([OP, BC], f32, tag="ps4")
            for l in (2, 3):
                a_out = apool.tile([128, KH, BC], fp8, tag="act")
                for j in range(JT):
                    wt = wpool.tile([128, KH, 128], fp8, tag="w")
                    nc.sync.dma_start(out=wt, in_=dr[f"w{l}"][j])
                    ps = pspool.tile([128, BC], f32, tag="ps")
                    for kt in range(0, KH, 2):
                        lhsT = wt[:, kt:kt + 2, :]
                        for b0 in range(0, BC, N0):
                            nc.tensor.matmul(
                                ps[:, b0:b0 + N0], lhsT,
                                a_in[:, kt:kt + 2, b0:b0 + N0],
                                start=(kt == 0), stop=(kt == KH - 2),
                                perf_mode=mybir.MatmulPerfMode.DoubleRow)
                    nc.scalar.activation(a_out[:, j, :], ps, AF.Sign,
                                         bias=c_sb[l][:, j:j + 1],
                                         scale=s_sb[l][:, j:j + 1])
                    # L4 pair for (j-3, j-2): delayed two j-tiles so the
                    # in-order PE queue never stalls on the ACT epilogue
                    if l == 3 and j % 2 == 1 and j >= 3:
                        kt = j - 3
                        for b0 in range(0, BC, N0):
                            nc.tensor.matmul(
                                ps4[:, b0:b0 + N0], w4t[:, kt:kt + 2, :],
                                a_out[:, kt:kt + 2, b0:b0 + N0],
                                start=(kt == 0), stop=False,
                                perf_mode=mybir.MatmulPerfMode.DoubleRow)
                if l == 3:
                    for kt in (KH - 2,):
                        for b0 in range(0, BC, N0):
                            nc.tensor.matmul(
                                ps4[:, b0:b0 + N0], w4t[:, kt:kt + 2, :],
                                a_out[:, kt:kt + 2, b0:b0 + N0],
                                start=False, stop=(kt == KH - 2),
                                perf_mode=mybir.MatmulPerfMode.DoubleRow)
                a_in = a_out

            y4 = small.tile([OP, BC], f32, tag="y4")
            nc.scalar.activation(y4, ps4, AF.Identity, bias=b4sb[:, 0:1],
                                 scale=1.0)

            # ---- transpose to [b, 10] and log_softmax ----
            for t in range(BC // 128):
                pst = pstpool.tile([128, OP], f32, tag="pst")
                nc.tensor.transpose(pst, y4[:, t * 128:(t + 1) * 128], ident)
                mxn = small.tile([128, 1], f32, tag="mx")
                nc.vector.reduce_max(mxn, pst[:, 0:OUT], axis=AX.X, negate=True)
                ex = small.tile([128, OUT], f32, tag="ex")
                sm = small.tile([128, 1], f32, tag="sm")
                nc.scalar.activation(ex, pst[:, 0:OUT], AF.Exp, bias=mxn,
                                     scale=1.0, accum_out=sm)
                lg = small.tile([128, 1], f32, tag="lg")
                nc.scalar.activation(lg, sm, AF.Ln)
                bias2 = small.tile([128, 1], f32, tag="b2")
                nc.vector.tensor_sub(bias2, mxn, lg)
                oo = small.tile([128, OUT], f32, tag="oo")
                nc.scalar.activation(oo, pst[:, 0:OUT], AF.Identity, bias=bias2,
                                     scale=1.0)
                row0 = b_off + t * 128
                nc.sync.dma_start(out=out_dram[row0:row0 + 128, :], in_=oo)

    nc.compile()
    return nc


def _sign(w):
    return np.where(w >= 0, np.float32(1.0), np.float32(-1.0))


def _prep_inputs(inputs):
    """Host-side: binarize/fold/retile weights, transpose+split x per core."""
    f32 = np.float32
    w = {i: _sign(inputs[f"w{i}"].astype(f32)) for i in (1, 2, 3, 4)}

    # layer 1 weights: [hi;lo] share sign(w1).T stacked twice, pad to 13 tiles
    w1t = np.zeros((KD * 128, H), f32)
    w1t[:D_IN] = w[1].T
    w1t[D_IN:2 * D_IN] = w[1].T
    w1b = np.ascontiguousarray(
        w1t.reshape(KD, 128, JT, 128).transpose(2, 1, 0, 3)).astype(np.float16)

    def hidden_w(wm):  # [4096, 4096] -> [jt, kp, kt, c] fp8
        return np.ascontiguousarray(
            wm.T.reshape(KH, 128, JT, 128).transpose(2, 1, 0, 3)
        ).astype(ml_dtypes.float8_e4m3)

    w2b, w3b = hidden_w(w[2]), hidden_w(w[3])

    # layer 4: [10, 4096] -> pad out to 16 -> [kp, kt, c] fp8
    w4t = np.zeros((H, OP), f32)
    w4t[:, :OUT] = w[4].T
    w4b = np.ascontiguousarray(
        w4t.reshape(KH, 128, OP).transpose(1, 0, 2)).astype(ml_dtypes.float8_e4m3)

    sc, cc = {}, {}
    for l in (1, 2, 3):
        g = inputs[f"g{l}"].astype(f32)
        v = inputs[f"v{l}"].astype(f32)
        bb = inputs[f"b{l}"].astype(f32)
        m = inputs[f"m{l}"].astype(f32)
        be = inputs[f"be{l}"].astype(f32)
        s = g / np.sqrt(v + np.float32(BN_EPS))
        c = (bb - m) * s + be
        sc[l] = np.ascontiguousarray(s.reshape(JT, 128).T)   # [128, JT]
        cc[l] = np.ascontiguousarray(c.reshape(JT, 128).T)
    b4 = np.zeros((OP, 1), f32)
    b4[:OUT, 0] = inputs["b4"].astype(f32)

    shared = {"w1": w1b, "w2": w2b, "w3": w3b, "w4": w4b, "b4": b4}
    for l in (1, 2, 3):
        shared[f"sc{l}"] = sc[l]
        shared[f"cc{l}"] = cc[l]

    x = inputs["x"].astype(f32)
    in_maps = []
    for c in range(N_CORES):
        xs = x[c * BS:(c + 1) * BS].T                     # [784, BS]
        hi = xs.astype(np.float16)
        lo = (xs - hi.astype(f32)).astype(np.float16)
        xp = np.zeros((KD * 128, BS), np.float16)
        xp[:D_IN] = hi
        xp[D_IN:2 * D_IN] = lo

        m = dict(shared)
        m["xt"] = np.ascontiguousarray(
            xp.reshape(KD, 128, BS).transpose(1, 0, 2))
        in_maps.append(m)
    return in_maps


def _run(inputs, trace=False):
    global _compiled
    from concourse.bass_utils import run_bass_kernel_spmd

    if _compiled is None:
        _compiled = _build_module()
    nc = _compiled
    in_maps = _prep_inputs(inputs)
    res = run_bass_kernel_spmd(nc, in_maps, core_ids=list(range(N_CORES)),
                               trace=trace)
    out = np.concatenate([res.results[c]["out"] for c in range(N_CORES)],
                         axis=0)
    return out.astype(np.float32), res


def kernel(**inputs):
    out, _ = _run(inputs, trace=False)
    return out
